# revision 16
# baseline (speedup 1.0000x reference)
"""VQ-VAE forward on 8 TRN2 NeuronCores (Bass/Tile), data-parallel over batch.

Per core (4 images):
  conv1 k4s2 1->16 : im2col from host-padded fp16 hi/lo x; K=72, M=128=(oc,8pix).
  conv2 k4s2 16->32: kx-span-8 expanded input, G=3 pixel groups, K=128,
                     4 ky-accumulating matmuls, M=96=(pix,oc).
  conv3 1x1 32->64 : 3 pixel-strip matmuls, K=32 row-strips.
  VQ: S = z.e - |e|^2/2 via K=65 fp16 hi/lo 3-pass matmuls (positions on
      partitions, codes on free) -> negated reduce-max -> ACT Sign(S-max)
      indicator -> 4x PE transpose -> codebook matmuls -> z_q directly.
  conv4 1x1; convt5/convt6 as phase-decomposed stride-1 convs; fp16, 1 pass.
  BatchNorm uses full-batch statistics: per-layer Sum/SumSq via ACT accum_out,
  AllReduce of (C,2) across the 8 cores, scale/bias broadcast by tiny matmul.
Encoder + VQ run at f32-equivalent precision (fp16 hi/lo splits); decoder fp16.
conv1/conv3/convt5 are computed twice (stats pass + apply pass) to save SBUF.
"""
import sys
import os

for _p in ('/opt/trn_rl_repo', '/root/.axon_site/_ro/trn_rl_repo'):
    if os.path.isdir(_p) and _p not in sys.path:
        sys.path.insert(0, _p)

import numpy as np
import ml_dtypes
from contextlib import ExitStack

import concourse.bass as bass
import concourse.bacc as bacc
import concourse.tile as tile
import concourse.tile_utils as tile_utils
import concourse.mybir as mybir
from concourse.bass_utils import run_bass_kernel_spmd

tile_utils.max_sbuf_usage = 207 * 1024

F32 = mybir.dt.float32
F16 = mybir.dt.float16
BF16 = mybir.dt.bfloat16
AF = mybir.ActivationFunctionType
ALU = mybir.AluOpType
AX = mybir.AxisListType
EPS = 1e-5

NCORES = 8
B = 4
NTOT = 32
CH = 512

G2, SP2, NG2 = 3, 8, 22          # conv2 grouping
C2COLS = NG2 * B * 64            # 5632 cols per strip, (g, img, Y) order
NPOS = 3 * C2COLS                # 16896
G5, SP5, NG5 = 2, 4, 32          # convt5
C5COLS = NG5 * B * 64            # 8192 per py
G6, SP6, NG6 = 6, 8, 22          # convt6
C6COLS = NG6 * B * 128           # 11264 per py

XPAD = 258
XIMC = 16 * B * 128              # conv1 im2col cols (g, img, r)
H1W = 140                        # h1pad width (1+128+1 used + rearrange slack)
H4W = 68
H5W = 140
PYD = [(0, 0), (0, -1), (1, 0), (1, 1)]   # (py, d) tap sets for convt


def _f16_split(a):
    hi = np.asarray(a, np.float64).astype(np.float16)
    lo = (np.asarray(a, np.float64) - hi.astype(np.float64)).astype(np.float16)
    return hi, lo


def _build_consts(enc, emb, dec):
    c = {}
    w1 = enc['w1'].astype(np.float64)
    l1 = np.zeros((72, 128), np.float64)
    for ky in range(4):
        for kx in range(18):
            for oc in range(16):
                for s in range(8):
                    q = kx - 2 * s
                    if 0 <= q <= 3:
                        l1[ky * 18 + kx, oc * 8 + s] = w1[oc, 0, ky, q]
    c['w1h'], c['w1l'] = _f16_split(l1)

    w2 = enc['w2'].astype(np.float64)
    l2 = np.zeros((128, 4 * 96), np.float64)
    for ky in range(4):
        for ci in range(16):
            for kxs in range(SP2):
                for pix in range(G2):
                    q = kxs - 2 * pix
                    if 0 <= q <= 3:
                        for oc in range(32):
                            l2[ci * SP2 + kxs, ky * 96 + pix * 32 + oc] = \
                                w2[oc, ci, ky, q]
    c['w2h'], c['w2l'] = _f16_split(l2)

    w3 = enc['w3'].astype(np.float64)[:, :, 0, 0]       # (64,32)
    l3 = np.zeros((96, 64), np.float64)
    for p in range(3):
        l3[32 * p:32 * p + 32, :] = w3.T
    c['w3h'], c['w3l'] = _f16_split(l3)

    E = emb.astype(np.float64)
    ex = np.zeros((65, 512), np.float64)
    ex[:64, :] = E.T
    e2 = (emb.astype(np.float32) ** 2).sum(1, dtype=np.float32).astype(np.float64)
    ex[64, :] = -0.5 * e2
    c['ehx'], c['elx'] = _f16_split(ex)

    Eh, El = _f16_split(emb)                     # (512,64) hi/lo
    er = np.zeros((128, 256), np.float16)
    erl = np.zeros((128, 256), np.float16)
    for b4 in range(4):
        er[:, b4 * 64:b4 * 64 + 64] = Eh[b4 * 128:(b4 + 1) * 128, :]
        erl[:, b4 * 64:b4 * 64 + 64] = El[b4 * 128:(b4 + 1) * 128, :]
    c['ef16'] = er
    c['ef16l'] = erl
    c['esum'] = (Eh.astype(np.float64) + El.astype(np.float64)) \
        .sum(0).astype(np.float32).reshape(64, 1)
    c['identf16'] = np.eye(128, dtype=np.float16)

    c['w4'] = dec['w4'].astype(np.float64)[:, :, 0, 0].T.astype(np.float16)

    w5 = dec['w5'].astype(np.float64)                   # (32,16,4,4)
    l5 = np.zeros((128, 4 * 64), np.float64)
    for si, (py, d) in enumerate(PYD):
        ky = py - 2 * d + 1
        for ci in range(32):
            for px in range(2):
                for pix in range(G5):
                    for e in ((0, -1) if px == 0 else (0, 1)):
                        kx = px - 2 * e + 1
                        kxs = pix + e + 1
                        if 0 <= kxs < SP5 and 0 <= kx <= 3 and 0 <= ky <= 3:
                            for oc in range(16):
                                l5[ci * SP5 + kxs,
                                   si * 64 + px * 32 + oc * 2 + pix] = \
                                    w5[ci, oc, ky, kx]
    c['w5'] = l5.astype(np.float16)

    w6 = dec['w6'].astype(np.float64)                   # (16,1,4,4)
    l6a = np.zeros((64, 4 * 12), np.float64)            # spans 0..3
    l6b = np.zeros((64, 4 * 12), np.float64)            # spans 4..7
    for si, (py, d) in enumerate(PYD):
        ky = py - 2 * d + 1
        for ci in range(16):
            for px in range(2):
                for pix in range(G6):
                    for e in ((0, -1) if px == 0 else (0, 1)):
                        kx = px - 2 * e + 1
                        kxs = pix + e + 1
                        if 0 <= kxs < SP6 and 0 <= kx <= 3 and 0 <= ky <= 3:
                            m = si * 12 + pix * 2 + px
                            if kxs < 4:
                                l6a[ci * 4 + kxs, m] = w6[ci, 0, ky, kx]
                            else:
                                l6b[ci * 4 + (kxs - 4), m] = w6[ci, 0, ky, kx]
    c['w6a'] = l6a.astype(np.float16)
    c['w6b'] = l6b.astype(np.float16)
    c['b6'] = np.full((12, 1), float(np.asarray(dec['b6']).reshape(-1)[0]), np.float32)

    for nm, g, b in [('gb1', enc['g1'], enc['be1']), ('gb2', enc['g2'], enc['be2']),
                     ('gb3', enc['g3'], enc['be3']), ('gb4', dec['g4'], dec['be4']),
                     ('gb5', dec['g5'], dec['be5'])]:
        c[nm] = np.stack([np.asarray(g), np.asarray(b)], 1).astype(np.float32)

    h1 = np.zeros((128, 16), np.float32)
    for oc in range(16):
        h1[oc * 8:oc * 8 + 8, oc] = 1
    c['hc1'], c['he1'] = h1, h1.T.copy()
    h2 = np.zeros((96, 32), np.float32)
    for pix in range(3):
        for oc in range(32):
            h2[pix * 32 + oc, oc] = 1
    c['hc2'], c['he2'] = h2, h2.T.copy()
    c['hc4'], c['he4'] = h2.copy(), h2.T.copy()         # same strip structure
    h5 = np.zeros((64, 16), np.float32)
    for px in range(2):
        for oc in range(16):
            for pix in range(G5):
                h5[px * 32 + oc * 2 + pix, oc] = 1
    c['hc5'], c['he5'] = h5, h5.T.copy()
    return c


CONST_SPECS = [
    ('w1h', [72, 128], F16), ('w1l', [72, 128], F16),
    ('w2h', [128, 384], F16), ('w2l', [128, 384], F16),
    ('w3h', [96, 64], F16), ('w3l', [96, 64], F16),
    ('ehx', [65, 512], F16), ('elx', [65, 512], F16),
    ('ef16', [128, 256], F16), ('ef16l', [128, 256], F16),
    ('esum', [64, 1], F32),
    ('identf16', [128, 128], F16),
    ('w4', [64, 32], F16), ('w5', [128, 256], F16),
    ('w6a', [64, 48], F16), ('w6b', [64, 48], F16), ('b6', [12, 1], F32),
    ('gb1', [16, 2], F32), ('gb2', [32, 2], F32), ('gb3', [64, 2], F32),
    ('gb4', [32, 2], F32), ('gb5', [16, 2], F32),
    ('hc1', [128, 16], F32), ('he1', [16, 128], F32),
    ('hc2', [96, 32], F32), ('he2', [32, 96], F32),
    ('hc4', [96, 32], F32), ('he4', [32, 96], F32),
    ('hc5', [64, 16], F32), ('he5', [16, 64], F32),
]


def build_nc():
    nc = bacc.Bacc("TRN2", target_bir_lowering=False, num_devices=NCORES)
    xih_in = nc.declare_dram_parameter("xih", [72, XIMC], F16, isOutput=False)
    xil_in = nc.declare_dram_parameter("xil", [72, XIMC], F16, isOutput=False)
    cin = {nm: nc.declare_dram_parameter(nm, shp, dt, isOutput=False)
           for nm, shp, dt in CONST_SPECS}
    ze_out = nc.declare_dram_parameter("ze", [64, NPOS], F32, isOutput=True)
    zq_out = nc.declare_dram_parameter("zq", [64, NPOS], F32, isOutput=True)
    xt_out = nc.declare_dram_parameter("xt", [12, 2 * NG6 * CH], F32, isOutput=True)

    with tile.TileContext(nc) as tc, ExitStack() as ctx:
        cpool = ctx.enter_context(tc.tile_pool(name="consts", bufs=1))
        sb = ctx.enter_context(tc.tile_pool(name="sb", bufs=1))
        rot = ctx.enter_context(tc.tile_pool(name="rot", bufs=2))
        rotb = ctx.enter_context(tc.tile_pool(name="rotb", bufs=1))
        ps = ctx.enter_context(tc.tile_pool(name="ps", bufs=2, space="PSUM"))
        pst = ctx.enter_context(tc.tile_pool(name="pst", bufs=2, space="PSUM"))
        psz = ctx.enter_context(tc.tile_pool(name="psz", bufs=2, space="PSUM"))
        dram = ctx.enter_context(tc.tile_pool(name="dram", bufs=1, space="DRAM"))

        ct = {}
        for nm, shp, dt in CONST_SPECS:
            t = cpool.tile(shp, dt, tag=f"c_{nm}")
            nc.sync.dma_start(t[:], cin[nm][:])
            ct[nm] = t
        zcol = cpool.tile([128, 1], F32, tag="zcol")
        nc.vector.memset(zcol[:], 0.0)
        epscol = cpool.tile([128, 1], F32, tag="epscol")
        nc.vector.memset(epscol[:], float(EPS))

        # ---------------- BN helpers ----------------
        def bn_allreduce(partials, nch, P, C, hc, he, gb, n_real, tag):
            w = rot.tile([128, 16], F32, tag="bnwork")
            sums = w[0:P, 0:2]
            nc.vector.tensor_reduce(sums, partials[:, :, 0:nch],
                                    axis=AX.X, op=ALU.add)
            if hc is not None:
                cps = pst.tile([C, 2], F32, tag="bcst")
                nc.tensor.matmul(cps[:], hc[:, 0:C], sums, start=True, stop=True)
                csums = w[0:C, 2:4]
                nc.scalar.activation(csums, cps[:], AF.Copy)
            else:
                csums = sums
            arin = dram.tile([C, 2], F32, tag=f"ari_{tag}")
            arout = dram.tile([C, 2], F32, tag=f"aro_{tag}")
            nc.sync.dma_start(arin[:], csums)
            nc.gpsimd.collective_compute(
                "AllReduce", ALU.add, replica_groups=[list(range(NCORES))],
                ins=[arin[:].opt()], outs=[arout[:].opt()])
            gsum = w[0:C, 4:6]
            nc.sync.dma_start(gsum, arout[:])
            inv_n = 1.0 / n_real
            m = w[0:C, 6:7]
            q = w[0:C, 7:8]
            v = w[0:C, 8:9]
            s = w[0:C, 9:10]
            r = w[0:C, 10:11]
            msc = w[0:C, 11:12]
            sb2 = w[0:C, 12:14]
            nc.vector.tensor_scalar(m, gsum[:, 0:1], inv_n, None, ALU.mult)
            nc.vector.tensor_scalar(q, gsum[:, 1:2], inv_n, None, ALU.mult)
            nc.vector.tensor_tensor(v, m, m, ALU.mult)
            nc.vector.tensor_tensor(v, q, v, ALU.subtract)
            nc.scalar.activation(s, v, AF.Sqrt, bias=epscol[0:C, :], scale=1.0)
            nc.vector.reciprocal(r, s)
            nc.vector.tensor_tensor(sb2[:, 0:1], gb[:, 0:1], r, ALU.mult)
            nc.vector.tensor_tensor(msc, m, sb2[:, 0:1], ALU.mult)
            nc.vector.tensor_tensor(sb2[:, 1:2], gb[:, 1:2], msc, ALU.subtract)
            out = sb.tile([P, 2], F32, tag=f"sbp_{tag}")
            if he is None:
                nc.vector.tensor_copy(out[:], sb2)
                return out
            bps = pst.tile([P, 2], F32, tag="bcst")
            nc.tensor.matmul(bps[:], he[0:C, 0:P], sb2, start=True, stop=True)
            nc.scalar.activation(out[:], bps[:], AF.Copy)
            return out

        def stats_from_psum(p, partials, ci, P):
            scr = rot.tile([128, CH], BF16, tag="scr")
            nc.scalar.activation(scr[0:P, :], p, AF.Copy,
                                 accum_out=partials[:, 0, ci:ci + 1])
            scr2 = rot.tile([128, CH], BF16, tag="scr")
            nc.scalar.activation(scr2[0:P, :], p, AF.Square, bias=zcol[0:P, :],
                                 accum_out=partials[:, 1, ci:ci + 1])

        # ================ conv1 ================
        xim_h = sb.tile([72, XIMC], F16, tag="tC")
        xim_l = sb.tile([72, XIMC], F16, tag="tD")
        nc.sync.dma_start(xim_h[:], xih_in[:])
        nc.sync.dma_start(xim_l[:], xil_in[:])
        NCH1 = 16
        part1 = sb.tile([128, 2, NCH1], F32, tag="part1")

        def conv1_mm(cc):
            p1 = ps.tile([128, CH], F32, tag="psA")
            sl = slice(cc * CH, cc * CH + CH)
            nc.tensor.matmul(p1[:], ct['w1h'][:], xim_h[:, sl], start=True, stop=False)
            nc.tensor.matmul(p1[:], ct['w1h'][:], xim_l[:, sl], start=False, stop=False)
            nc.tensor.matmul(p1[:], ct['w1l'][:], xim_h[:, sl], start=False, stop=True)
            return p1

        for cc in range(NCH1):
            stats_from_psum(conv1_mm(cc)[:], part1, cc, 128)
        sbp1 = bn_allreduce(part1, NCH1, 128, 16, ct['hc1'], ct['he1'],
                            ct['gb1'], float(NTOT * 128 * 128), "l1")
        h1h = sb.tile([128, 16, B, 128], F16, tag="tH")
        h1l = sb.tile([128, 16, B, 128], F16, tag="tI")
        h1h_f = h1h[:].rearrange("p g i r -> p (g i r)")
        h1l_f = h1l[:].rearrange("p g i r -> p (g i r)")
        for cc in range(NCH1):
            p1 = conv1_mm(cc)
            sl = slice(cc * CH, cc * CH + CH)
            yc = rotb.tile([128, CH], F32, tag="ych")
            nc.scalar.activation(yc[:], p1[:], AF.Relu,
                                 bias=sbp1[:, 1:2], scale=sbp1[:, 0:1])
            nc.vector.tensor_copy(h1h_f[:, sl], yc[:])
            yh32 = rotb.tile([128, CH], F32, tag="h32")
            nc.vector.tensor_copy(yh32[:], h1h_f[:, sl])
            nc.gpsimd.tensor_tensor(h1l_f[:, sl], yc[:], yh32[:], ALU.subtract)

        # h1 padded, transposed storage: (128=(16c,4i,2rh), x140, y66)
        # rh0 holds padded rows 0..65, rh1 holds 64..129 (overlap 64,65)
        h1ph = sb.tile([128, H1W, 66], F16, tag="tA")
        h1pl = sb.tile([128, H1W, 66], F16, tag="tB")
        for t in (h1ph, h1pl):
            nc.vector.memset(t[:], 0.0)
        for s in range(8):
            for img in range(B):
                for rh in range(2):
                    r0, r1, yoff = (0, 65, 1) if rh == 0 else (63, 128, 0)
                    nr = r1 - r0
                    for t, hsrc in ((h1ph, h1h), (h1pl, h1l)):
                        sr = hsrc[:].rearrange("(o s) g i r -> o s g i r", s=8)
                        sr = sr[:, s, :, img, r0:r1]            # (16o,16g,nr)
                        d = t[:].rearrange("(c q) x y -> c q x y", q=8)
                        d = d[:, img * 2 + rh, 1:129, yoff:yoff + nr]
                        d = d.rearrange("c (g k) y -> c g k y", k=8)[:, :, s, :]
                        nc.sync.dma_start(d, sr)

        # ================ conv2 ================
        # ex2: (128=(16c,8kxs), img, g22, y130); val = h1pad[c, x=6g+kxs, y]
        ex2h = sb.tile([128, B, NG2, 130], F16, tag="tC")
        ex2l = sb.tile([128, B, NG2, 130], F16, tag="tD")
        for kxs in range(SP2):
            for img in range(B):
                for rh in range(2):
                    ys = 0 if rh == 0 else 64
                    for dt_, st_ in ((ex2h, h1ph), (ex2l, h1pl)):
                        d = dt_[:].rearrange("(c k) i g y -> c k i g y", k=SP2)
                        d = d[:, kxs, img, :, ys:ys + 66]
                        sr = st_[:].rearrange("(c q) x y -> c q x y", q=8)
                        sr = sr[:, img * 2 + rh, kxs:kxs + 132, :]
                        sr = sr.rearrange("c (g k) y -> c g k y", k=6)[:, :, 0, :]
                        nc.sync.dma_start(d, sr)
        NCH2 = 11
        raw2 = sb.tile([96, NCH2 * CH], F32, tag="tE")
        part2 = sb.tile([96, 2, NCH2], F32, tag="part2")
        ex2h5 = ex2h[:].rearrange("p i g (t o) -> p i g t o", o=2)
        ex2l5 = ex2l[:].rearrange("p i g (t o) -> p i g t o", o=2)
        for cc in range(NCH2):
            p2 = ps.tile([96, CH], F32, tag="psA")
            mms = []
            for ky in range(4):
                t0, o = ky // 2, ky % 2
                rhs4 = ex2h5[:, :, :, t0:t0 + 64, o].transpose([0, 2, 1, 3])
                rhs4l = ex2l5[:, :, :, t0:t0 + 64, o].transpose([0, 2, 1, 3])
                wh = ct['w2h'][:, ky * 96:ky * 96 + 96]
                wl = ct['w2l'][:, ky * 96:ky * 96 + 96]
                mms += [(wh, rhs4), (wh, rhs4l), (wl, rhs4)]
            for i, (w, r4) in enumerate(mms):
                nc.tensor.matmul(p2[:], w, r4[:, 2 * cc:2 * cc + 2, :, :],
                                 start=(i == 0), stop=(i == len(mms) - 1))
            if cc == NCH2 - 1:
                nc.vector.memset(p2[32:64, 256:512], 0.0)
                nc.vector.memset(p2[64:96, 256:512], 0.0)
            stats_from_psum(p2[:], part2, cc, 96)
            nc.scalar.activation(raw2[:, cc * CH:cc * CH + CH], p2[:], AF.Copy)
        sbp2 = bn_allreduce(part2, NCH2, 96, 32, ct['hc2'], ct['he2'],
                            ct['gb2'], float(NTOT * 64 * 64), "l2")
        h2h = sb.tile([96, C2COLS], F16, tag="tH")
        h2l = sb.tile([96, C2COLS], F16, tag="tI")
        for cc in range(NCH2):
            sl = slice(cc * CH, cc * CH + CH)
            yc = rotb.tile([128, CH], F32, tag="ych")
            nc.scalar.activation(yc[0:96, :], raw2[:, sl], AF.Relu,
                                 bias=sbp2[:, 1:2], scale=sbp2[:, 0:1])
            nc.vector.tensor_copy(h2h[:, sl], yc[0:96, :])
            yh32 = rotb.tile([128, CH], F32, tag="h32")
            nc.vector.tensor_copy(yh32[0:96, :], h2h[:, sl])
            nc.gpsimd.tensor_tensor(h2l[:, sl], yc[0:96, :], yh32[0:96, :],
                                    ALU.subtract)

        # ================ conv3 + VQ ================
        NCH3 = 11
        part3 = sb.tile([64, 2, 3 * NCH3], F32, tag="part3")

        def conv3_mm(pix, cc):
            p3 = ps.tile([64, CH], F32, tag="psA")
            lh = ct['w3h'][32 * pix:32 * pix + 32, :]
            ll = ct['w3l'][32 * pix:32 * pix + 32, :]
            rh = h2h[32 * pix:32 * pix + 32, cc * CH:cc * CH + CH]
            rl = h2l[32 * pix:32 * pix + 32, cc * CH:cc * CH + CH]
            nc.tensor.matmul(p3[:], lh, rh, start=True, stop=False)
            nc.tensor.matmul(p3[:], lh, rl, start=False, stop=False)
            nc.tensor.matmul(p3[:], ll, rh, start=False, stop=True)
            if pix >= 1 and cc == NCH3 - 1:
                nc.vector.memset(p3[:, 256:512], 0.0)
            return p3

        for pix in range(3):
            for cc in range(NCH3):
                stats_from_psum(conv3_mm(pix, cc)[:], part3, pix * NCH3 + cc, 64)
        sbp3 = bn_allreduce(part3, 3 * NCH3, 64, 64, None, None,
                            ct['gb3'], float(NTOT * 64 * 64), "l3")

        zqh = sb.tile([64, NPOS], F16, tag="tG")
        for pix in range(3):
            for cc in range(NCH3):
                p3 = conv3_mm(pix, cc)
                csl = slice(pix * C2COLS + cc * CH, pix * C2COLS + cc * CH + CH)
                zev = rotb.tile([64, CH], F32, tag="zev")
                nc.scalar.activation(zev[:], p3[:], AF.Identity,
                                     bias=sbp3[:, 1:2], scale=sbp3[:, 0:1])
                nc.sync.dma_start(bass.AP(ze_out, csl.start,
                                          [[NPOS, 64], [1, CH]]), zev[:])
                zh = rot.tile([65, CH], F16, tag="zh")
                zl = rot.tile([65, CH], F16, tag="zl")
                nc.vector.memset(zh[64:65, :], 1.0)
                nc.vector.memset(zl[64:65, :], 0.0)
                nc.vector.tensor_copy(zh[0:64, :], zev[:])
                zh32 = rotb.tile([64, CH], F32, tag="h32")
                nc.vector.tensor_copy(zh32[:], zh[0:64, :])
                nc.gpsimd.tensor_tensor(zl[0:64, :], zev[:], zh32[:], ALU.subtract)
                zqv = rotb.tile([64, CH], F32, tag="zqv")
                for qq in range(4):
                    qs = slice(qq * 128, qq * 128 + 128)
                    sp = ps.tile([128, 512], F32, tag="psA")
                    nc.tensor.matmul(sp[:], zh[:, qs], ct['ehx'][:],
                                     start=True, stop=False)
                    nc.tensor.matmul(sp[:], zh[:, qs], ct['elx'][:],
                                     start=False, stop=False)
                    nc.tensor.matmul(sp[:], zl[:, qs], ct['ehx'][:],
                                     start=False, stop=True)
                    nm = rot.tile([128, 1], F32, tag="negmax")
                    nc.vector.tensor_reduce(nm[:], sp[:], axis=AX.X, op=ALU.max,
                                            negate=True)
                    ind = rot.tile([128, 512], F16, tag="ind")
                    nc.scalar.activation(ind[:], sp[:], AF.Sign,
                                         bias=nm[:], scale=1.0)
                    tp = pst.tile([128, 512], F16, tag="indT")
                    for b4 in range(4):
                        nc.tensor.matmul(tp[:, b4 * 128:b4 * 128 + 128],
                                         ind[:, b4 * 128:b4 * 128 + 128],
                                         ct['identf16'][:], is_transpose=True,
                                         skip_group_check=(b4 > 0))
                    inds = rot.tile([128, 512], F16, tag="indS")
                    nc.scalar.activation(inds[:], tp[:], AF.Copy)
                    zp = psz.tile([64, 128], F32, tag="psB")
                    for b4 in range(4):
                        nc.tensor.matmul(zp[:],
                                         ct['ef16'][:, b4 * 64:b4 * 64 + 64],
                                         inds[:, b4 * 128:b4 * 128 + 128],
                                         start=(b4 == 0), stop=False)
                        nc.tensor.matmul(zp[:],
                                         ct['ef16l'][:, b4 * 64:b4 * 64 + 64],
                                         inds[:, b4 * 128:b4 * 128 + 128],
                                         start=False, stop=(b4 == 3))
                    nc.scalar.activation(zqv[:, qs], zp[:], AF.Identity,
                                         bias=ct['esum'][:], scale=1.0)
                nc.sync.dma_start(bass.AP(zq_out, csl.start,
                                          [[NPOS, 64], [1, CH]]), zqv[:])
                nc.vector.tensor_copy(zqh[:, csl], zqv[:])

        # ================ conv4 ================
        NCH4 = 11
        raw4 = sb.tile([96, NCH4 * CH], F32, tag="tE")
        part4 = sb.tile([96, 2, 3 * NCH4], F32, tag="part4")
        nc.vector.memset(part4[:], 0.0)
        for pix in range(3):
            for cc in range(NCH4):
                p4 = ps.tile([96, CH], F32, tag="psA")
                pp = p4[32 * pix:32 * pix + 32, :]
                rhs = zqh[:, pix * C2COLS + cc * CH: pix * C2COLS + cc * CH + CH]
                nc.tensor.matmul(pp, ct['w4'][:], rhs, start=True, stop=True)
                if pix >= 1 and cc == NCH4 - 1:
                    nc.vector.memset(p4[32 * pix:32 * pix + 32, 256:512], 0.0)
                ci = pix * NCH4 + cc
                prt = part4[32 * pix:32 * pix + 32, :, :]
                scr = rot.tile([128, CH], BF16, tag="scr")
                nc.scalar.activation(scr[32 * pix:32 * pix + 32, :], pp, AF.Copy,
                                     accum_out=prt[:, 0, ci:ci + 1])
                scr2 = rot.tile([128, CH], BF16, tag="scr")
                nc.scalar.activation(scr2[32 * pix:32 * pix + 32, :], pp,
                                     AF.Square, bias=zcol[0:32, :],
                                     accum_out=prt[:, 1, ci:ci + 1])
                nc.scalar.activation(raw4[32 * pix:32 * pix + 32,
                                          cc * CH:cc * CH + CH], pp, AF.Copy)
        sbp4 = bn_allreduce(part4, 3 * NCH4, 96, 32, ct['hc4'], ct['he4'],
                            ct['gb4'], float(NTOT * 64 * 64), "l4")
        h4 = sb.tile([96, C2COLS], F16, tag="tH")
        for cc in range(NCH4):
            sl = slice(cc * CH, cc * CH + CH)
            nc.scalar.activation(h4[:, sl], raw4[:, sl], AF.Relu,
                                 bias=sbp4[:, 1:2], scale=sbp4[:, 0:1])
        # h4 padded transposed: (128=(32c,4i), x68, y66)
        h4p = sb.tile([128, H4W, 66], F16, tag="tA")
        nc.vector.memset(h4p[:], 0.0)
        for pix in range(3):
            ng = NG2 if pix == 0 else NG2 - 1
            for img in range(B):
                sr = h4[32 * pix:32 * pix + 32, :]
                sr = sr.rearrange("c (g i y) -> c g i y", g=NG2, i=B)
                sr = sr[:, 0:ng, img, :]                     # (32c, ng, 64Y)
                d = h4p[:].rearrange("(c q) x y -> c q x y", q=B)
                d = d[:, img, 1:67, 1:65]
                d = d.rearrange("c (g k) y -> c g k y", k=3)[:, 0:ng, pix, :]
                nc.sync.dma_start(d, sr)

        # ================ convt5 ================
        # ex5: (128=(32c,4kxs), img, g32, y66); val = h4pad[c, x=2g+kxs, y]
        ex5 = sb.tile([128, B, NG5, 66], F16, tag="tB")
        for kxs in range(SP5):
            for img in range(B):
                d = ex5[:].rearrange("(c k) i g y -> c k i g y", k=SP5)
                d = d[:, kxs, img]
                sr = h4p[:].rearrange("(c q) x y -> c q x y", q=B)[:, img]
                sr = sr[:, kxs:kxs + 64, :]
                sr = sr.rearrange("c (g k) y -> c g k y", k=2)[:, :, 0, :]
                nc.sync.dma_start(d, sr)
        # psum chunks: (g, img4, yfull128); py phases interleave psum columns
        NCH5 = NG5
        part5 = sb.tile([64, 2, NCH5], F32, tag="part5")

        def convt5_mm(cc):
            p5 = ps.tile([64, CH], F32, tag="psA")
            p5v = p5[:].rearrange("m (i t o) -> m i t o", i=B, o=2)
            for py in range(2):
                sets = [si for si, (pyy, _) in enumerate(PYD) if pyy == py]
                for j, si in enumerate(sets):
                    d = PYD[si][1]
                    rhs = ex5[:, :, cc:cc + 1, 1 + d:1 + d + 64]
                    rhs = rhs.transpose([0, 2, 1, 3])    # (p, g1, i, Y)
                    nc.tensor.matmul(p5v[:, :, :, py],
                                     ct['w5'][:, si * 64:si * 64 + 64], rhs,
                                     start=(j == 0), stop=(j == len(sets) - 1),
                                     skip_group_check=(py == 1))
            return p5

        for cc in range(NCH5):
            stats_from_psum(convt5_mm(cc)[:], part5, cc, 64)
        sbp5 = bn_allreduce(part5, NCH5, 64, 16, ct['hc5'], ct['he5'],
                            ct['gb5'], float(NTOT * 128 * 128), "l5")
        h5 = sb.tile([64, NG5 * B * 128], F16, tag="tG")
        for cc in range(NCH5):
            p5 = convt5_mm(cc)
            nc.scalar.activation(h5[:, cc * CH:cc * CH + CH], p5[:], AF.Relu,
                                 bias=sbp5[:, 1:2], scale=sbp5[:, 0:1])
        # h5 padded transposed: (128=(16c,4i,2rh), x140, y66)
        h5p = sb.tile([128, H5W, 66], F16, tag="tA")
        nc.vector.memset(h5p[:], 0.0)
        for px in range(2):
            for pix in range(G5):
                xb = 2 * pix + px + 1
                goff, xoff = xb // 4, xb % 4
                for img in range(B):
                    for rh in range(2):
                        ys, yoff = (0, 1) if rh == 0 else (63, 0)
                        sr = h5[:].rearrange("(x o p) n -> x o p n", x=2, o=16)
                        sr = sr[px, :, pix, :]
                        sr = sr.rearrange("c (g i y) -> c g i y", g=NG5, i=B)
                        sr = sr[:, :, img, ys:ys + 65]       # (16c, 32g, 65y)
                        d = h5p[:].rearrange("(c q) x y -> c q x y", q=8)
                        d = d[:, img * 2 + rh]
                        d = d.rearrange("c (g k) y -> c g k y", k=4)
                        d = d[:, goff:goff + NG5, xoff, yoff:yoff + 65]
                        nc.sync.dma_start(d, sr)

        # ================ convt6 + sigmoid ================
        # ex6: (64=(16c,4ko), img, g22, y130) x2 (spans 0-3 / 4-7)
        ex6a = sb.tile([64, B, NG6, 130], F16, tag="tD")
        ex6b = sb.tile([64, B, NG6, 130], F16, tag="tB")
        for kxs in range(SP6):
            t, ko = (ex6a, kxs) if kxs < 4 else (ex6b, kxs - 4)
            for img in range(B):
                for rh in range(2):
                    ys = 0 if rh == 0 else 64
                    d = t[:].rearrange("(c k) i g y -> c k i g y", k=4)
                    d = d[:, ko, img, :, ys:ys + 66]
                    sr = h5p[:].rearrange("(c q) x y -> c q x y", q=8)
                    sr = sr[:, img * 2 + rh, kxs:kxs + 132, :]
                    sr = sr.rearrange("c (g k) y -> c g k y", k=6)[:, :, 0, :]
                    nc.sync.dma_start(d, sr)
        for g6 in range(NG6):
            for ih in range(2):
                p6 = ps.tile([12, CH], F32, tag="psA")
                p6v = p6[:].rearrange("m (i t o) -> m i t o", i=2, o=2)
                for py in range(2):
                    sets = [si for si, (pyy, _) in enumerate(PYD) if pyy == py]
                    k = 0
                    for si in sets:
                        d = PYD[si][1]
                        for t, w in ((ex6a, ct['w6a']), (ex6b, ct['w6b'])):
                            rhs = t[:, 2 * ih:2 * ih + 2, g6:g6 + 1,
                                    1 + d:1 + d + 128]
                            rhs = rhs.transpose([0, 2, 1, 3])  # (p,g1,i2,Y)
                            nc.tensor.matmul(p6v[:, :, :, py],
                                             w[:, si * 12:si * 12 + 12], rhs,
                                             start=(k == 0), stop=(k == 3),
                                             skip_group_check=(py == 1))
                            k += 1
                sg = rot.tile([12, CH], F32, tag="sig")
                nc.scalar.activation(sg[:], p6[:], AF.Sigmoid,
                                     bias=ct['b6'][:], scale=1.0)
                off = (g6 * 2 + ih) * CH
                nc.sync.dma_start(bass.AP(xt_out, off,
                                          [[2 * NG6 * CH, 12], [1, CH]]), sg[:])
    nc.finalize()
    return nc


_CACHED = {}


def kernel(x, enc, emb, dec):
    x = np.asarray(x)
    emb = np.asarray(emb)
    enc = {k: np.asarray(v) for k, v in enc.items()}
    dec = {k: np.asarray(v) for k, v in dec.items()}
    consts = _build_consts(enc, emb, dec)
    if 'nc' not in _CACHED:
        _CACHED['nc'] = build_nc()
    nc = _CACHED['nc']
    in_maps = []
    for core in range(NCORES):
        xs = x[core * B:(core + 1) * B, 0]
        xp = np.zeros((B, XPAD, XPAD), np.float64)
        xp[:, 1:257, 1:257] = xs
        # im2col: (72=(ky,18span), g16, img, r128): val = xp[i, 2r+ky, 16g+kx]
        st = xp.strides
        v = np.lib.stride_tricks.as_strided(
            xp, shape=(4, 18, 16, B, 128),
            strides=(st[1], st[2], 16 * st[2], st[0], 2 * st[1]))
        v = v.reshape(72, 16, B, 128)
        xh, xl = _f16_split(v)
        m = {'xih': np.ascontiguousarray(xh), 'xil': np.ascontiguousarray(xl)}
        for nm, shp, dt in CONST_SPECS:
            vv = consts[nm]
            assert list(vv.shape) == shp, (nm, vv.shape, shp)
            m[nm] = np.ascontiguousarray(vv)
        in_maps.append(m)
    trace = bool(int(os.environ.get("KERNEL_TRACE", "0")))
    res = run_bass_kernel_spmd(nc, in_maps, list(range(NCORES)), trace=trace)
    kernel.last_exec_time_ns = res.exec_time_ns
    # ---- host unshuffle ----
    ze = np.empty((NTOT, 64, 64, 64), np.float32)
    zq = np.empty((NTOT, 64, 64, 64), np.float32)
    xt = np.empty((NTOT, 1, 256, 256), np.float32)
    for core in range(NCORES):
        r = res.results[core]
        for pix in range(3):
            ng = NG2 if pix == 0 else NG2 - 1
            for nm, dst in (('ze', ze), ('zq', zq)):
                v = r[nm][:, pix * C2COLS:pix * C2COLS + ng * B * 64]
                v = v.reshape(64, ng, B, 64)          # (c, g, img, Y)
                dst[core * B:(core + 1) * B, :, :, pix::3][:, :, :, 0:ng] = \
                    v.transpose(2, 0, 3, 1)
        v = r['xt'].reshape(12, NG6, 2, 2, 256)        # (m,(g,ih,i2,y))
        for g in range(NG6):
            for m in range(12):
                xx = 12 * g + m
                if xx >= 256:
                    continue
                xt[core * B:(core + 1) * B, 0, :, xx] = \
                    v[m, g].reshape(4, 256)
    return xt, ze, zq


# revision 17
# speedup vs baseline: 1.1515x; 1.1515x over previous
"""VQ-VAE forward on 8 TRN2 NeuronCores (Bass/Tile), data-parallel over batch.

Per core (4 images):
  conv1 k4s2 1->16 : im2col from host-padded fp16 hi/lo x; K=72, M=128=(oc,8pix).
  conv2 k4s2 16->32: kx-span-8 expanded input, G=3 pixel groups, K=128,
                     4 ky-accumulating matmuls, M=96=(pix,oc).
  conv3 1x1 32->64 : 3 pixel-strip matmuls, K=32 row-strips.
  VQ: S = z.e - |e|^2/2 via K=65 fp16 hi/lo 3-pass matmuls (positions on
      partitions, codes on free) -> negated reduce-max -> ACT Sign(S-max)
      indicator -> 4x PE transpose -> codebook matmuls -> z_q directly.
  conv4 1x1; convt5/convt6 as phase-decomposed stride-1 convs; fp16, 1 pass.
  BatchNorm uses full-batch statistics: per-layer Sum/SumSq via ACT accum_out,
  AllReduce of (C,2) across the 8 cores, scale/bias broadcast by tiny matmul.
Encoder + VQ run at f32-equivalent precision (fp16 hi/lo splits); decoder fp16.
conv1/conv3/convt5 are computed twice (stats pass + apply pass) to save SBUF.
"""
import sys
import os

for _p in ('/opt/trn_rl_repo', '/root/.axon_site/_ro/trn_rl_repo'):
    if os.path.isdir(_p) and _p not in sys.path:
        sys.path.insert(0, _p)

import numpy as np
import ml_dtypes
from contextlib import ExitStack

import concourse.bass as bass
import concourse.bacc as bacc
import concourse.tile as tile
import concourse.tile_utils as tile_utils
import concourse.mybir as mybir
from concourse.bass_utils import run_bass_kernel_spmd

tile_utils.max_sbuf_usage = 207 * 1024

F32 = mybir.dt.float32
F16 = mybir.dt.float16
BF16 = mybir.dt.bfloat16
AF = mybir.ActivationFunctionType
ALU = mybir.AluOpType
AX = mybir.AxisListType
EPS = 1e-5

NCORES = 8
B = 4
NTOT = 32
CH = 512

G2, SP2, NG2 = 3, 8, 22          # conv2 grouping
C2COLS = NG2 * B * 64            # 5632 cols per strip, (g, img, Y) order
NPOS = 3 * C2COLS                # 16896
G5, SP5, NG5 = 2, 4, 32          # convt5
C5COLS = NG5 * B * 64            # 8192 per py
G6, SP6, NG6 = 6, 8, 22          # convt6
C6COLS = NG6 * B * 128           # 11264 per py

XPAD = 258
XIMC = 16 * B * 128              # conv1 im2col cols (g, img, r)
H1W = 140                        # h1pad width (1+128+1 used + rearrange slack)
H4W = 68
H5W = 140
PYD = [(0, 0), (0, -1), (1, 0), (1, 1)]   # (py, d) tap sets for convt


def _f16_split(a):
    hi = np.asarray(a, np.float64).astype(np.float16)
    lo = (np.asarray(a, np.float64) - hi.astype(np.float64)).astype(np.float16)
    return hi, lo


def _build_consts(enc, emb, dec):
    c = {}
    w1 = enc['w1'].astype(np.float64)
    l1 = np.zeros((72, 128), np.float64)
    for ky in range(4):
        for kx in range(18):
            for oc in range(16):
                for s in range(8):
                    q = kx - 2 * s
                    if 0 <= q <= 3:
                        l1[ky * 18 + kx, oc * 8 + s] = w1[oc, 0, ky, q]
    c['w1h'], c['w1l'] = _f16_split(l1)

    w2 = enc['w2'].astype(np.float64)
    l2 = np.zeros((128, 4 * 96), np.float64)
    for ky in range(4):
        for ci in range(16):
            for kxs in range(SP2):
                for pix in range(G2):
                    q = kxs - 2 * pix
                    if 0 <= q <= 3:
                        for oc in range(32):
                            l2[ci * SP2 + kxs, ky * 96 + pix * 32 + oc] = \
                                w2[oc, ci, ky, q]
    c['w2h'], c['w2l'] = _f16_split(l2)

    w3 = enc['w3'].astype(np.float64)[:, :, 0, 0]       # (64,32)
    l3 = np.zeros((96, 64), np.float64)
    for p in range(3):
        l3[32 * p:32 * p + 32, :] = w3.T
    c['w3h'], c['w3l'] = _f16_split(l3)

    E = emb.astype(np.float64)
    ex = np.zeros((65, 512), np.float64)
    ex[:64, :] = E.T
    e2 = (emb.astype(np.float32) ** 2).sum(1, dtype=np.float32).astype(np.float64)
    ex[64, :] = -0.5 * e2
    c['ehx'], c['elx'] = _f16_split(ex)

    Eh, El = _f16_split(emb)                     # (512,64) hi/lo
    er = np.zeros((128, 256), np.float16)
    erl = np.zeros((128, 256), np.float16)
    for b4 in range(4):
        er[:, b4 * 64:b4 * 64 + 64] = Eh[b4 * 128:(b4 + 1) * 128, :]
        erl[:, b4 * 64:b4 * 64 + 64] = El[b4 * 128:(b4 + 1) * 128, :]
    c['ef16'] = er
    c['ef16l'] = erl
    c['esum'] = (Eh.astype(np.float64) + El.astype(np.float64)) \
        .sum(0).astype(np.float32).reshape(64, 1)
    c['identf16'] = np.eye(128, dtype=np.float16)

    c['w4'] = dec['w4'].astype(np.float64)[:, :, 0, 0].T.astype(np.float16)

    w5 = dec['w5'].astype(np.float64)                   # (32,16,4,4)
    l5 = np.zeros((128, 4 * 64), np.float64)
    for si, (py, d) in enumerate(PYD):
        ky = py - 2 * d + 1
        for ci in range(32):
            for px in range(2):
                for pix in range(G5):
                    for e in ((0, -1) if px == 0 else (0, 1)):
                        kx = px - 2 * e + 1
                        kxs = pix + e + 1
                        if 0 <= kxs < SP5 and 0 <= kx <= 3 and 0 <= ky <= 3:
                            for oc in range(16):
                                l5[ci * SP5 + kxs,
                                   si * 64 + px * 32 + oc * 2 + pix] = \
                                    w5[ci, oc, ky, kx]
    c['w5'] = l5.astype(np.float16)

    w6 = dec['w6'].astype(np.float64)                   # (16,1,4,4)
    l6a = np.zeros((64, 4 * 12), np.float64)            # spans 0..3
    l6b = np.zeros((64, 4 * 12), np.float64)            # spans 4..7
    for si, (py, d) in enumerate(PYD):
        ky = py - 2 * d + 1
        for ci in range(16):
            for px in range(2):
                for pix in range(G6):
                    for e in ((0, -1) if px == 0 else (0, 1)):
                        kx = px - 2 * e + 1
                        kxs = pix + e + 1
                        if 0 <= kxs < SP6 and 0 <= kx <= 3 and 0 <= ky <= 3:
                            m = si * 12 + pix * 2 + px
                            if kxs < 4:
                                l6a[ci * 4 + kxs, m] = w6[ci, 0, ky, kx]
                            else:
                                l6b[ci * 4 + (kxs - 4), m] = w6[ci, 0, ky, kx]
    c['w6a'] = l6a.astype(np.float16)
    c['w6b'] = l6b.astype(np.float16)
    c['b6'] = np.full((12, 1), float(np.asarray(dec['b6']).reshape(-1)[0]), np.float32)

    for nm, g, b in [('gb1', enc['g1'], enc['be1']), ('gb2', enc['g2'], enc['be2']),
                     ('gb3', enc['g3'], enc['be3']), ('gb4', dec['g4'], dec['be4']),
                     ('gb5', dec['g5'], dec['be5'])]:
        c[nm] = np.stack([np.asarray(g), np.asarray(b)], 1).astype(np.float32)

    h1 = np.zeros((128, 16), np.float32)
    for oc in range(16):
        h1[oc * 8:oc * 8 + 8, oc] = 1
    c['hc1'], c['he1'] = h1, h1.T.copy()
    h2 = np.zeros((96, 32), np.float32)
    for pix in range(3):
        for oc in range(32):
            h2[pix * 32 + oc, oc] = 1
    c['hc2'], c['he2'] = h2, h2.T.copy()
    c['hc4'], c['he4'] = h2.copy(), h2.T.copy()         # same strip structure
    h5 = np.zeros((64, 16), np.float32)
    for px in range(2):
        for oc in range(16):
            for pix in range(G5):
                h5[px * 32 + oc * 2 + pix, oc] = 1
    c['hc5'], c['he5'] = h5, h5.T.copy()
    return c


CONST_SPECS = [
    ('w1h', [72, 128], F16), ('w1l', [72, 128], F16),
    ('w2h', [128, 384], F16), ('w2l', [128, 384], F16),
    ('w3h', [96, 64], F16), ('w3l', [96, 64], F16),
    ('ehx', [65, 512], F16), ('elx', [65, 512], F16),
    ('ef16', [128, 256], F16), ('ef16l', [128, 256], F16),
    ('esum', [64, 1], F32),
    ('identf16', [128, 128], F16),
    ('w4', [64, 32], F16), ('w5', [128, 256], F16),
    ('w6a', [64, 48], F16), ('w6b', [64, 48], F16), ('b6', [12, 1], F32),
    ('gb1', [16, 2], F32), ('gb2', [32, 2], F32), ('gb3', [64, 2], F32),
    ('gb4', [32, 2], F32), ('gb5', [16, 2], F32),
    ('hc1', [128, 16], F32), ('he1', [16, 128], F32),
    ('hc2', [96, 32], F32), ('he2', [32, 96], F32),
    ('hc4', [96, 32], F32), ('he4', [32, 96], F32),
    ('hc5', [64, 16], F32), ('he5', [16, 64], F32),
]


def build_nc():
    nc = bacc.Bacc("TRN2", target_bir_lowering=False, num_devices=NCORES)
    xih_in = nc.declare_dram_parameter("xih", [72, XIMC], F16, isOutput=False)
    xil_in = nc.declare_dram_parameter("xil", [72, XIMC], F16, isOutput=False)
    cin = {nm: nc.declare_dram_parameter(nm, shp, dt, isOutput=False)
           for nm, shp, dt in CONST_SPECS}
    ze_out = nc.declare_dram_parameter("ze", [64, NPOS], F32, isOutput=True)
    zq_out = nc.declare_dram_parameter("zq", [64, NPOS], F32, isOutput=True)
    xt_out = nc.declare_dram_parameter("xt", [12, 2 * NG6 * CH], F32, isOutput=True)

    with tile.TileContext(nc) as tc, ExitStack() as ctx:
        cpool = ctx.enter_context(tc.tile_pool(name="consts", bufs=1))
        sb = ctx.enter_context(tc.tile_pool(name="sb", bufs=1))
        rot = ctx.enter_context(tc.tile_pool(name="rot", bufs=2))
        rotb = ctx.enter_context(tc.tile_pool(name="rotb", bufs=1))
        ps = ctx.enter_context(tc.tile_pool(name="ps", bufs=3, space="PSUM"))
        pst = ctx.enter_context(tc.tile_pool(name="pst", bufs=2, space="PSUM"))
        psz = ctx.enter_context(tc.tile_pool(name="psz", bufs=2, space="PSUM"))
        psc = ctx.enter_context(tc.tile_pool(name="psc", bufs=1, space="PSUM"))
        dram = ctx.enter_context(tc.tile_pool(name="dram", bufs=1, space="DRAM"))

        ct = {}
        for nm, shp, dt in CONST_SPECS:
            t = cpool.tile(shp, dt, tag=f"c_{nm}")
            nc.sync.dma_start(t[:], cin[nm][:])
            ct[nm] = t
        zcol = cpool.tile([128, 1], F32, tag="zcol")
        nc.vector.memset(zcol[:], 0.0)
        epscol = cpool.tile([128, 1], F32, tag="epscol")
        nc.vector.memset(epscol[:], float(EPS))

        # ---------------- BN helpers ----------------
        def bn_allreduce(partials, nch, P, C, hc, he, gb, n_real, tag):
            w = rot.tile([128, 16], F32, tag="bnwork")
            sums = w[0:P, 0:2]
            nc.vector.tensor_reduce(sums, partials[:, :, 0:nch],
                                    axis=AX.X, op=ALU.add)
            if hc is not None:
                cps = psc.tile([C, 2], F32, tag="bcst")
                nc.tensor.matmul(cps[:], hc[:, 0:C], sums, start=True, stop=True)
                csums = w[0:C, 2:4]
                nc.scalar.activation(csums, cps[:], AF.Copy)
            else:
                csums = sums
            arin = dram.tile([C, 2], F32, tag=f"ari_{tag}")
            arout = dram.tile([C, 2], F32, tag=f"aro_{tag}")
            nc.sync.dma_start(arin[:], csums)
            nc.gpsimd.collective_compute(
                "AllReduce", ALU.add, replica_groups=[list(range(NCORES))],
                ins=[arin[:].opt()], outs=[arout[:].opt()])
            gsum = w[0:C, 4:6]
            nc.sync.dma_start(gsum, arout[:])
            inv_n = 1.0 / n_real
            m = w[0:C, 6:7]
            q = w[0:C, 7:8]
            v = w[0:C, 8:9]
            s = w[0:C, 9:10]
            r = w[0:C, 10:11]
            msc = w[0:C, 11:12]
            sb2 = w[0:C, 12:14]
            nc.vector.tensor_scalar(m, gsum[:, 0:1], inv_n, None, ALU.mult)
            nc.vector.tensor_scalar(q, gsum[:, 1:2], inv_n, None, ALU.mult)
            nc.vector.tensor_tensor(v, m, m, ALU.mult)
            nc.vector.tensor_tensor(v, q, v, ALU.subtract)
            nc.scalar.activation(s, v, AF.Sqrt, bias=epscol[0:C, :], scale=1.0)
            nc.vector.reciprocal(r, s)
            nc.vector.tensor_tensor(sb2[:, 0:1], gb[:, 0:1], r, ALU.mult)
            nc.vector.tensor_tensor(msc, m, sb2[:, 0:1], ALU.mult)
            nc.vector.tensor_tensor(sb2[:, 1:2], gb[:, 1:2], msc, ALU.subtract)
            out = sb.tile([P, 2], F32, tag=f"sbp_{tag}")
            if he is None:
                nc.vector.tensor_copy(out[:], sb2)
                return out
            bps = psc.tile([P, 2], F32, tag="bcst")
            nc.tensor.matmul(bps[:], he[0:C, 0:P], sb2, start=True, stop=True)
            nc.scalar.activation(out[:], bps[:], AF.Copy)
            return out

        def stats_from_psum(p, partials, ci, P):
            scr = rot.tile([128, CH], BF16, tag="scr")
            nc.scalar.activation(scr[0:P, :], p, AF.Copy,
                                 accum_out=partials[:, 0, ci:ci + 1])
            scr2 = rot.tile([128, CH], BF16, tag="scr")
            nc.scalar.activation(scr2[0:P, :], p, AF.Square, bias=zcol[0:P, :],
                                 accum_out=partials[:, 1, ci:ci + 1])

        # ================ conv1 ================
        xim_h = sb.tile([72, XIMC], F16, tag="tC")
        xim_l = sb.tile([72, XIMC], F16, tag="tD")
        nc.sync.dma_start(xim_h[:], xih_in[:])
        nc.sync.dma_start(xim_l[:], xil_in[:])
        NCH1 = 16
        part1 = sb.tile([128, 2, NCH1], F32, tag="part1")

        def conv1_mm(cc):
            p1 = ps.tile([128, CH], F32, tag="psA")
            sl = slice(cc * CH, cc * CH + CH)
            nc.tensor.matmul(p1[:], ct['w1h'][:], xim_h[:, sl], start=True, stop=False)
            nc.tensor.matmul(p1[:], ct['w1h'][:], xim_l[:, sl], start=False, stop=False)
            nc.tensor.matmul(p1[:], ct['w1l'][:], xim_h[:, sl], start=False, stop=True)
            return p1

        for cc in range(NCH1):
            stats_from_psum(conv1_mm(cc)[:], part1, cc, 128)
        sbp1 = bn_allreduce(part1, NCH1, 128, 16, ct['hc1'], ct['he1'],
                            ct['gb1'], float(NTOT * 128 * 128), "l1")
        h1h = sb.tile([128, 16, B, 128], F16, tag="tH")
        h1l = sb.tile([128, 16, B, 128], F16, tag="tI")
        h1h_f = h1h[:].rearrange("p g i r -> p (g i r)")
        h1l_f = h1l[:].rearrange("p g i r -> p (g i r)")
        for cc in range(NCH1):
            p1 = conv1_mm(cc)
            sl = slice(cc * CH, cc * CH + CH)
            yc = rotb.tile([128, CH], F32, tag="ych")
            nc.scalar.activation(yc[:], p1[:], AF.Relu,
                                 bias=sbp1[:, 1:2], scale=sbp1[:, 0:1])
            nc.vector.tensor_copy(h1h_f[:, sl], yc[:])
            nc.gpsimd.tensor_tensor(h1l_f[:, sl], yc[:], h1h_f[:, sl],
                                    ALU.subtract)

        # h1 padded, transposed storage: (128=(16c,4i,2rh), x140, y66)
        # rh0 holds padded rows 0..65, rh1 holds 64..129 (overlap 64,65)
        h1ph = sb.tile([128, H1W, 66], F16, tag="tA")
        h1pl = sb.tile([128, H1W, 66], F16, tag="tB")
        for t in (h1ph, h1pl):
            nc.vector.memset(t[:], 0.0)
        for s in range(8):
            for img in range(B):
                for rh in range(2):
                    r0, r1, yoff = (0, 65, 1) if rh == 0 else (63, 128, 0)
                    nr = r1 - r0
                    for t, hsrc in ((h1ph, h1h), (h1pl, h1l)):
                        sr = hsrc[:].rearrange("(o s) g i r -> o s g i r", s=8)
                        sr = sr[:, s, :, img, r0:r1]            # (16o,16g,nr)
                        d = t[:].rearrange("(c q) x y -> c q x y", q=8)
                        d = d[:, img * 2 + rh, 1:129, yoff:yoff + nr]
                        d = d.rearrange("c (g k) y -> c g k y", k=8)[:, :, s, :]
                        nc.sync.dma_start(d, sr)

        # ================ conv2 ================
        # ex2: (128=(16c,8kxs), img, g22, y130); val = h1pad[c, x=6g+kxs, y]
        ex2h = sb.tile([128, B, NG2, 130], F16, tag="tC")
        ex2l = sb.tile([128, B, NG2, 130], F16, tag="tD")
        for kxs in range(SP2):
            for img in range(B):
                for rh in range(2):
                    ys = 0 if rh == 0 else 64
                    for dt_, st_ in ((ex2h, h1ph), (ex2l, h1pl)):
                        d = dt_[:].rearrange("(c k) i g y -> c k i g y", k=SP2)
                        d = d[:, kxs, img, :, ys:ys + 66]
                        sr = st_[:].rearrange("(c q) x y -> c q x y", q=8)
                        sr = sr[:, img * 2 + rh, kxs:kxs + 132, :]
                        sr = sr.rearrange("c (g k) y -> c g k y", k=6)[:, :, 0, :]
                        nc.sync.dma_start(d, sr)
        NCH2 = 11
        raw2 = sb.tile([96, NCH2 * CH], F32, tag="tE")
        part2 = sb.tile([96, 2, NCH2], F32, tag="part2")
        ex2h5 = ex2h[:].rearrange("p i g (t o) -> p i g t o", o=2)
        ex2l5 = ex2l[:].rearrange("p i g (t o) -> p i g t o", o=2)
        for cc in range(NCH2):
            p2 = ps.tile([96, CH], F32, tag="psA")
            mms = []
            for ky in range(4):
                t0, o = ky // 2, ky % 2
                rhs4 = ex2h5[:, :, :, t0:t0 + 64, o].transpose([0, 2, 1, 3])
                rhs4l = ex2l5[:, :, :, t0:t0 + 64, o].transpose([0, 2, 1, 3])
                wh = ct['w2h'][:, ky * 96:ky * 96 + 96]
                wl = ct['w2l'][:, ky * 96:ky * 96 + 96]
                mms += [(wh, rhs4), (wh, rhs4l), (wl, rhs4)]
            for i, (w, r4) in enumerate(mms):
                nc.tensor.matmul(p2[:], w, r4[:, 2 * cc:2 * cc + 2, :, :],
                                 start=(i == 0), stop=(i == len(mms) - 1))
            if cc == NCH2 - 1:
                nc.vector.memset(p2[32:64, 256:512], 0.0)
                nc.vector.memset(p2[64:96, 256:512], 0.0)
            nc.scalar.activation(raw2[:, cc * CH:cc * CH + CH], p2[:], AF.Copy,
                                 accum_out=part2[:, 0, cc:cc + 1])
            scr2 = rot.tile([128, CH], BF16, tag="scr")
            nc.scalar.activation(scr2[0:96, :], p2[:], AF.Square,
                                 bias=zcol[0:96, :],
                                 accum_out=part2[:, 1, cc:cc + 1])
        sbp2 = bn_allreduce(part2, NCH2, 96, 32, ct['hc2'], ct['he2'],
                            ct['gb2'], float(NTOT * 64 * 64), "l2")
        h2h = sb.tile([96, C2COLS], F16, tag="tH")
        h2l = sb.tile([96, C2COLS], F16, tag="tI")
        for cc in range(NCH2):
            sl = slice(cc * CH, cc * CH + CH)
            yc = rotb.tile([128, CH], F32, tag="ych")
            nc.scalar.activation(yc[0:96, :], raw2[:, sl], AF.Relu,
                                 bias=sbp2[:, 1:2], scale=sbp2[:, 0:1])
            nc.vector.tensor_copy(h2h[:, sl], yc[0:96, :])
            nc.gpsimd.tensor_tensor(h2l[:, sl], yc[0:96, :], h2h[:, sl],
                                    ALU.subtract)

        # ================ conv3 + VQ ================
        NCH3 = 11
        part3 = sb.tile([64, 2, 3 * NCH3], F32, tag="part3")

        def conv3_mm(pix, cc):
            p3 = ps.tile([64, CH], F32, tag="psA")
            lh = ct['w3h'][32 * pix:32 * pix + 32, :]
            ll = ct['w3l'][32 * pix:32 * pix + 32, :]
            rh = h2h[32 * pix:32 * pix + 32, cc * CH:cc * CH + CH]
            rl = h2l[32 * pix:32 * pix + 32, cc * CH:cc * CH + CH]
            nc.tensor.matmul(p3[:], lh, rh, start=True, stop=False)
            nc.tensor.matmul(p3[:], lh, rl, start=False, stop=False)
            nc.tensor.matmul(p3[:], ll, rh, start=False, stop=True)
            if pix >= 1 and cc == NCH3 - 1:
                nc.vector.memset(p3[:, 256:512], 0.0)
            return p3

        for pix in range(3):
            for cc in range(NCH3):
                stats_from_psum(conv3_mm(pix, cc)[:], part3, pix * NCH3 + cc, 64)
        sbp3 = bn_allreduce(part3, 3 * NCH3, 64, 64, None, None,
                            ct['gb3'], float(NTOT * 64 * 64), "l3")

        zqh = sb.tile([64, NPOS], F16, tag="tG")
        for pix in range(3):
            for cc in range(NCH3):
                p3 = conv3_mm(pix, cc)
                csl = slice(pix * C2COLS + cc * CH, pix * C2COLS + cc * CH + CH)
                zev = rotb.tile([64, CH], F32, tag="zev")
                nc.scalar.activation(zev[:], p3[:], AF.Identity,
                                     bias=sbp3[:, 1:2], scale=sbp3[:, 0:1])
                nc.sync.dma_start(bass.AP(ze_out, csl.start,
                                          [[NPOS, 64], [1, CH]]), zev[:])
                zh = rot.tile([65, CH], F16, tag="zh")
                zl = rot.tile([65, CH], F16, tag="zl")
                nc.vector.memset(zh[64:65, :], 1.0)
                nc.vector.memset(zl[64:65, :], 0.0)
                nc.vector.tensor_copy(zh[0:64, :], zev[:])
                nc.gpsimd.tensor_tensor(zl[0:64, :], zev[:], zh[0:64, :],
                                        ALU.subtract)
                zqv = rotb.tile([64, CH], F32, tag="zqv")
                for qq in range(4):
                    qs = slice(qq * 128, qq * 128 + 128)
                    sp = ps.tile([128, 512], F32, tag="psA")
                    nc.tensor.matmul(sp[:], zh[:, qs], ct['ehx'][:],
                                     start=True, stop=False)
                    nc.tensor.matmul(sp[:], zh[:, qs], ct['elx'][:],
                                     start=False, stop=False)
                    nc.tensor.matmul(sp[:], zl[:, qs], ct['ehx'][:],
                                     start=False, stop=True)
                    nm = rot.tile([128, 1], F32, tag="negmax")
                    nc.vector.tensor_reduce(nm[:], sp[:], axis=AX.X, op=ALU.max,
                                            negate=True)
                    ind = rot.tile([128, 512], F16, tag="ind")
                    nc.scalar.activation(ind[:], sp[:], AF.Sign,
                                         bias=nm[:], scale=1.0)
                    tp = pst.tile([128, 512], F16, tag="indT")
                    for b4 in range(4):
                        nc.tensor.matmul(tp[:, b4 * 128:b4 * 128 + 128],
                                         ind[:, b4 * 128:b4 * 128 + 128],
                                         ct['identf16'][:], is_transpose=True,
                                         skip_group_check=(b4 > 0))
                    inds = rot.tile([128, 512], F16, tag="indS")
                    nc.scalar.activation(inds[:], tp[:], AF.Copy)
                    zp = psz.tile([64, 128], F32, tag="psB")
                    for b4 in range(4):
                        nc.tensor.matmul(zp[:],
                                         ct['ef16'][:, b4 * 64:b4 * 64 + 64],
                                         inds[:, b4 * 128:b4 * 128 + 128],
                                         start=(b4 == 0), stop=False)
                        nc.tensor.matmul(zp[:],
                                         ct['ef16l'][:, b4 * 64:b4 * 64 + 64],
                                         inds[:, b4 * 128:b4 * 128 + 128],
                                         start=False, stop=(b4 == 3))
                    nc.scalar.activation(zqv[:, qs], zp[:], AF.Identity,
                                         bias=ct['esum'][:], scale=1.0)
                nc.sync.dma_start(bass.AP(zq_out, csl.start,
                                          [[NPOS, 64], [1, CH]]), zqv[:])
                nc.vector.tensor_copy(zqh[:, csl], zqv[:])

        # ================ conv4 ================
        NCH4 = 11
        raw4 = sb.tile([96, NCH4 * CH], F32, tag="tE")
        part4 = sb.tile([96, 2, 3 * NCH4], F32, tag="part4")
        nc.vector.memset(part4[:], 0.0)
        for pix in range(3):
            for cc in range(NCH4):
                p4 = ps.tile([96, CH], F32, tag="psA")
                pp = p4[32 * pix:32 * pix + 32, :]
                rhs = zqh[:, pix * C2COLS + cc * CH: pix * C2COLS + cc * CH + CH]
                nc.tensor.matmul(pp, ct['w4'][:], rhs, start=True, stop=True)
                if pix >= 1 and cc == NCH4 - 1:
                    nc.vector.memset(p4[32 * pix:32 * pix + 32, 256:512], 0.0)
                ci = pix * NCH4 + cc
                prt = part4[32 * pix:32 * pix + 32, :, :]
                nc.scalar.activation(raw4[32 * pix:32 * pix + 32,
                                          cc * CH:cc * CH + CH], pp, AF.Copy,
                                     accum_out=prt[:, 0, ci:ci + 1])
                scr2 = rot.tile([128, CH], BF16, tag="scr")
                nc.scalar.activation(scr2[32 * pix:32 * pix + 32, :], pp,
                                     AF.Square, bias=zcol[0:32, :],
                                     accum_out=prt[:, 1, ci:ci + 1])
        sbp4 = bn_allreduce(part4, 3 * NCH4, 96, 32, ct['hc4'], ct['he4'],
                            ct['gb4'], float(NTOT * 64 * 64), "l4")
        h4 = sb.tile([96, C2COLS], F16, tag="tH")
        for cc in range(NCH4):
            sl = slice(cc * CH, cc * CH + CH)
            nc.scalar.activation(h4[:, sl], raw4[:, sl], AF.Relu,
                                 bias=sbp4[:, 1:2], scale=sbp4[:, 0:1])
        # h4 padded transposed: (128=(32c,4i), x68, y66)
        h4p = sb.tile([128, H4W, 66], F16, tag="tA")
        nc.vector.memset(h4p[:], 0.0)
        for pix in range(3):
            ng = NG2 if pix == 0 else NG2 - 1
            for img in range(B):
                sr = h4[32 * pix:32 * pix + 32, :]
                sr = sr.rearrange("c (g i y) -> c g i y", g=NG2, i=B)
                sr = sr[:, 0:ng, img, :]                     # (32c, ng, 64Y)
                d = h4p[:].rearrange("(c q) x y -> c q x y", q=B)
                d = d[:, img, 1:67, 1:65]
                d = d.rearrange("c (g k) y -> c g k y", k=3)[:, 0:ng, pix, :]
                nc.sync.dma_start(d, sr)

        # ================ convt5 ================
        # ex5: (128=(32c,4kxs), img, g32, y66); val = h4pad[c, x=2g+kxs, y]
        ex5 = sb.tile([128, B, NG5, 66], F16, tag="tB")
        for kxs in range(SP5):
            for img in range(B):
                d = ex5[:].rearrange("(c k) i g y -> c k i g y", k=SP5)
                d = d[:, kxs, img]
                sr = h4p[:].rearrange("(c q) x y -> c q x y", q=B)[:, img]
                sr = sr[:, kxs:kxs + 64, :]
                sr = sr.rearrange("c (g k) y -> c g k y", k=2)[:, :, 0, :]
                nc.sync.dma_start(d, sr)
        # psum chunks: (g, img4, yfull128); py phases interleave psum columns
        NCH5 = NG5
        part5 = sb.tile([64, 2, NCH5], F32, tag="part5")

        def convt5_mm(cc):
            p5 = ps.tile([64, CH], F32, tag="psA")
            p5v = p5[:].rearrange("m (i t o) -> m i t o", i=B, o=2)
            for py in range(2):
                sets = [si for si, (pyy, _) in enumerate(PYD) if pyy == py]
                for j, si in enumerate(sets):
                    d = PYD[si][1]
                    rhs = ex5[:, :, cc:cc + 1, 1 + d:1 + d + 64]
                    rhs = rhs.transpose([0, 2, 1, 3])    # (p, g1, i, Y)
                    nc.tensor.matmul(p5v[:, :, :, py],
                                     ct['w5'][:, si * 64:si * 64 + 64], rhs,
                                     start=(j == 0), stop=(j == len(sets) - 1),
                                     skip_group_check=(py == 1))
            return p5

        for cc in range(NCH5):
            stats_from_psum(convt5_mm(cc)[:], part5, cc, 64)
        sbp5 = bn_allreduce(part5, NCH5, 64, 16, ct['hc5'], ct['he5'],
                            ct['gb5'], float(NTOT * 128 * 128), "l5")
        h5 = sb.tile([64, NG5 * B * 128], F16, tag="tG")
        for cc in range(NCH5):
            p5 = convt5_mm(cc)
            nc.scalar.activation(h5[:, cc * CH:cc * CH + CH], p5[:], AF.Relu,
                                 bias=sbp5[:, 1:2], scale=sbp5[:, 0:1])
        # h5 padded transposed: (128=(16c,4i,2rh), x140, y66)
        h5p = sb.tile([128, H5W, 66], F16, tag="tA")
        nc.vector.memset(h5p[:], 0.0)
        for px in range(2):
            for pix in range(G5):
                xb = 2 * pix + px + 1
                goff, xoff = xb // 4, xb % 4
                for img in range(B):
                    for rh in range(2):
                        ys, yoff = (0, 1) if rh == 0 else (63, 0)
                        sr = h5[:].rearrange("(x o p) n -> x o p n", x=2, o=16)
                        sr = sr[px, :, pix, :]
                        sr = sr.rearrange("c (g i y) -> c g i y", g=NG5, i=B)
                        sr = sr[:, :, img, ys:ys + 65]       # (16c, 32g, 65y)
                        d = h5p[:].rearrange("(c q) x y -> c q x y", q=8)
                        d = d[:, img * 2 + rh]
                        d = d.rearrange("c (g k) y -> c g k y", k=4)
                        d = d[:, goff:goff + NG5, xoff, yoff:yoff + 65]
                        nc.sync.dma_start(d, sr)

        # ================ convt6 + sigmoid ================
        # ex6: (64=(16c,4ko), img, g22, y130) x2 (spans 0-3 / 4-7)
        ex6a = sb.tile([64, B, NG6, 130], F16, tag="tD")
        ex6b = sb.tile([64, B, NG6, 130], F16, tag="tB")
        for kxs in range(SP6):
            t, ko = (ex6a, kxs) if kxs < 4 else (ex6b, kxs - 4)
            for img in range(B):
                for rh in range(2):
                    ys = 0 if rh == 0 else 64
                    d = t[:].rearrange("(c k) i g y -> c k i g y", k=4)
                    d = d[:, ko, img, :, ys:ys + 66]
                    sr = h5p[:].rearrange("(c q) x y -> c q x y", q=8)
                    sr = sr[:, img * 2 + rh, kxs:kxs + 132, :]
                    sr = sr.rearrange("c (g k) y -> c g k y", k=6)[:, :, 0, :]
                    nc.sync.dma_start(d, sr)
        for g6 in range(NG6):
            for ih in range(2):
                p6 = ps.tile([12, CH], F32, tag="psA")
                p6v = p6[:].rearrange("m (i t o) -> m i t o", i=2, o=2)
                for py in range(2):
                    sets = [si for si, (pyy, _) in enumerate(PYD) if pyy == py]
                    k = 0
                    for si in sets:
                        d = PYD[si][1]
                        for t, w in ((ex6a, ct['w6a']), (ex6b, ct['w6b'])):
                            rhs = t[:, 2 * ih:2 * ih + 2, g6:g6 + 1,
                                    1 + d:1 + d + 128]
                            rhs = rhs.transpose([0, 2, 1, 3])  # (p,g1,i2,Y)
                            nc.tensor.matmul(p6v[:, :, :, py],
                                             w[:, si * 12:si * 12 + 12], rhs,
                                             start=(k == 0), stop=(k == 3),
                                             skip_group_check=(py == 1))
                            k += 1
                sg = rot.tile([12, CH], F32, tag="sig")
                nc.scalar.activation(sg[:], p6[:], AF.Sigmoid,
                                     bias=ct['b6'][:], scale=1.0)
                off = (g6 * 2 + ih) * CH
                nc.sync.dma_start(bass.AP(xt_out, off,
                                          [[2 * NG6 * CH, 12], [1, CH]]), sg[:])
    nc.finalize()
    return nc


_CACHED = {}


def kernel(x, enc, emb, dec):
    x = np.asarray(x)
    emb = np.asarray(emb)
    enc = {k: np.asarray(v) for k, v in enc.items()}
    dec = {k: np.asarray(v) for k, v in dec.items()}
    consts = _build_consts(enc, emb, dec)
    if 'nc' not in _CACHED:
        _CACHED['nc'] = build_nc()
    nc = _CACHED['nc']
    in_maps = []
    for core in range(NCORES):
        xs = x[core * B:(core + 1) * B, 0]
        xp = np.zeros((B, XPAD, XPAD), np.float64)
        xp[:, 1:257, 1:257] = xs
        # im2col: (72=(ky,18span), g16, img, r128): val = xp[i, 2r+ky, 16g+kx]
        st = xp.strides
        v = np.lib.stride_tricks.as_strided(
            xp, shape=(4, 18, 16, B, 128),
            strides=(st[1], st[2], 16 * st[2], st[0], 2 * st[1]))
        v = v.reshape(72, 16, B, 128)
        xh, xl = _f16_split(v)
        m = {'xih': np.ascontiguousarray(xh), 'xil': np.ascontiguousarray(xl)}
        for nm, shp, dt in CONST_SPECS:
            vv = consts[nm]
            assert list(vv.shape) == shp, (nm, vv.shape, shp)
            m[nm] = np.ascontiguousarray(vv)
        in_maps.append(m)
    trace = bool(int(os.environ.get("KERNEL_TRACE", "0")))
    res = run_bass_kernel_spmd(nc, in_maps, list(range(NCORES)), trace=trace)
    kernel.last_exec_time_ns = res.exec_time_ns
    # ---- host unshuffle ----
    ze = np.empty((NTOT, 64, 64, 64), np.float32)
    zq = np.empty((NTOT, 64, 64, 64), np.float32)
    xt = np.empty((NTOT, 1, 256, 256), np.float32)
    for core in range(NCORES):
        r = res.results[core]
        for pix in range(3):
            ng = NG2 if pix == 0 else NG2 - 1
            for nm, dst in (('ze', ze), ('zq', zq)):
                v = r[nm][:, pix * C2COLS:pix * C2COLS + ng * B * 64]
                v = v.reshape(64, ng, B, 64)          # (c, g, img, Y)
                dst[core * B:(core + 1) * B, :, :, pix::3][:, :, :, 0:ng] = \
                    v.transpose(2, 0, 3, 1)
        v = r['xt'].reshape(12, NG6, 2, 2, 256)        # (m,(g,ih,i2,y))
        for g in range(NG6):
            for m in range(12):
                xx = 12 * g + m
                if xx >= 256:
                    continue
                xt[core * B:(core + 1) * B, 0, :, xx] = \
                    v[m, g].reshape(4, 256)
    return xt, ze, zq
